# revision 9
# baseline (speedup 1.0000x reference)
"""Multi-head attention kernel for Trainium2, 8 NeuronCores.

Problem (hardcoded shapes): B=4, S=2048, E=1024, H=16, DH=64.
  q/k/v = einsum('bse,hed->bhsd', x, W{q,k,v}) + b{q,k,v}
  attn  = softmax(q k^T / sqrt(DH)) v
  out   = concat_heads(attn) @ Wo^T + bo

Sharding: core c -> (batch b = c//2, head-half hh = c%2, i.e. heads
8*hh..8*hh+7).  Each core computes a [S, E] partial of its batch's output
(its 512 columns of concat against the matching 512 rows of Wo^T); the host
sums the two partials per batch and adds the bias vector.

Bias algebra (exactness): softmax over keys t of
  (q_s+bq)·(k_t+bk) = q_s·k_t + bq·k_t + [q_s·bk + bq·bk]
The bracketed terms are constant in t and cancel in softmax, so bk is
dropped entirely; bq is folded into the qT copy.  bv commutes through the
attention average, so bv's contribution to the output is the constant
vector bv_cat @ Wo^T, added on the host together with bo.

Per-core dataflow (all PE inputs bf16; PSUM accumulation fp32):
  xT   [e=128 x 8, s=2048]  host-pretransposed, DMA'd directly
  qT/kT[j=128, s=2048]      per pair p: Wp^T @ xT (+bq on the q copy)
  v    [t, (h,65)]          xT @ Wv, with a fused ones column per head
  scores  [t=128, 1024]     per (pair, sc, tb): two matmuls, one per head
                            (cols 0:512 head 2p, 512: head 2p+1), both rhs
                            qT[*, sc*512:+512]
  ex                        ACT Exp(scale=1/8) -> bf16 SBUF
  attn+sums [s=128, 65]     lhsT = ex column-block (N=65!), rhs = v slice,
                            accumulated over the 16 t-blocks per (h, sb)
  normalize                 DVE recip + per-partition tensor_scalar_mul
                            -> concat [s, f] bf16
  concatT [f, s]            PE transpose (bf16), after all heads of an sb
  out_partial [s, e]        concatT as lhsT, Wo^T as rhs, DMA'd from PSUM

The stream is one software-pipelined generator over (pair, sc, tb) with
scores->exp one slot ahead of attn-V, and fillers (projection chunks, v
blocks, output chunks) injected on an ACT-cadence cycle budget with
deadline forcing, so both the PE and the scalar engine stay busy.
"""

import os
import sys

for _p in ("/opt/trn_rl_repo", "/root/.axon_site/_ro/trn_rl_repo"):
    if os.path.isdir(_p) and _p not in sys.path:
        sys.path.insert(0, _p)
        break

from contextlib import ExitStack

import numpy as np
import ml_dtypes

import concourse.bass as bass
import concourse.tile as tile
import concourse.mybir as mybir
from concourse import bacc, bass_utils

B, S, E, H, DH = 4, 2048, 1024, 16, 64
HPC = 8           # heads per core
JW = HPC * DH     # 512, per-core qkv width
N_CORES = 8
SB = S // 128     # 16 s-blocks / t-blocks
EB = E // 128     # 8 e-blocks
SC = S // 512     # 4 s-chunks
F32 = mybir.dt.float32
BF16 = mybir.dt.bfloat16
Exp = mybir.ActivationFunctionType.Exp
MULT = mybir.AluOpType.mult

# ACT slot cadence in PE cycles: one [128,1024] exp takes ~1038ns busy
# (~2600 PE cycles at 2.4GHz); fillers are injected against this budget.
SLOT_BUDGET = 2600
STREAM_COST = 1544      # scores (1024) + attn-V (520) per slot


def _emit(tc, aps, ctx):
    nc = tc.nc
    xt_d, wqk_d, wv_d, wo_d, bq_d, id_d, out_d = aps

    def pool(**kw):
        return ctx.enter_context(tc.tile_pool(**kw))

    const = pool(name="const", bufs=1)
    xTp = pool(name="xT", bufs=1)
    vxp = pool(name="vext", bufs=1)
    wqk = pool(name="wqk", bufs=2)
    qkp = pool(name="qk", bufs=2)
    exp_p = pool(name="ex", bufs=3)
    ccp = pool(name="cc", bufs=1)
    nrm = pool(name="nrm", bufs=2)
    outp = pool(name="outs", bufs=3)
    ps_sc = pool(name="ps_sc", bufs=2, space="PSUM")   # scores [128,1024] x2 = 4 banks
    ps_ac = pool(name="ps_ac", bufs=1, space="PSUM")   # acc [128,4,65] x2 tags = 2 banks
    ps_sm = pool(name="ps_sm", bufs=2, space="PSUM")   # proj/outs/transposes = 2 banks

    # ---- constants ----
    ident = const.tile([128, 128], BF16)
    nc.sync.dma_start(ident[:], id_d[:])
    bq_sb = const.tile([128, 4], F32)
    nc.sync.dma_start(bq_sb[:], bq_d[:])
    wv_sb = const.tile([128, EB, JW], BF16)
    wo_sb = const.tile([128, 4, E], BF16)

    xT = xTp.tile([128, EB, S], BF16)
    vext = vxp.tile([128, SB, HPC, DH + 1], BF16)
    nc.gpsimd.memset(vext[:, :, :, DH:DH + 1], 1.0)
    concat = ccp.tile([128, SB, JW], BF16)    # [s, (sb), f]
    concatT = ccp.tile([128, 4, S], BF16)     # [f, (fb), s]

    def dma_xt(eb, sc):
        nc.sync.dma_start(xT[:, eb, sc * 512:(sc + 1) * 512],
                          xt_d[:, eb * S + sc * 512: eb * S + (sc + 1) * 512])

    def dma_wv():
        nc.sync.dma_start(
            wv_sb[:].rearrange("p eb j -> p (eb j)"), wv_d[:])

    def dma_wo():
        nc.sync.dma_start(
            wo_sb[:].rearrange("p fb e -> p (fb e)"), wo_d[:])

    pair_w = {}

    def dma_pair_weights(p):
        wq_t = wqk.tile([128, EB, 128], BF16, tag="wq")
        nc.sync.dma_start(wq_t[:].rearrange("p eb j -> p (eb j)"),
                          wqk_d[p * 128:(p + 1) * 128, :])
        wk_t = wqk.tile([128, EB, 128], BF16, tag="wk")
        nc.sync.dma_start(wk_t[:].rearrange("p eb j -> p (eb j)"),
                          wqk_d[(4 + p) * 128:(5 + p) * 128, :])
        pair_w[p] = (wq_t, wk_t)

    pair_qk = {}

    def alloc_pair_qk(p):
        qT = qkp.tile([128, S], BF16, tag="qT")
        kT = qkp.tile([128, S], BF16, tag="kT")
        pair_qk[p] = (qT, kT)

    def proj_chunk(p, which, sc):
        """q or k projection for pair p, s-chunk sc: 8 matmuls + copy."""
        wq_t, wk_t = pair_w[p]
        qT, kT = pair_qk[p]
        w_t, dst = (wq_t, qT) if which == "q" else (wk_t, kT)
        pq = ps_sm.tile([128, 512], F32, tag="ps_sm",
                        name=f"p{which}{p}_{sc}")
        for eb in range(EB):
            nc.tensor.matmul(pq[:], w_t[:, eb, :],
                             xT[:, eb, sc * 512:(sc + 1) * 512],
                             start=(eb == 0), stop=(eb == EB - 1))
        if which == "q":
            nc.vector.tensor_scalar_add(
                dst[:, sc * 512:(sc + 1) * 512], pq[:], bq_sb[:, p:p + 1])
        else:
            nc.vector.tensor_copy(dst[:, sc * 512:(sc + 1) * 512], pq[:])

    def v_chunk(tb):
        """v projection for t-block tb (all 8 heads) into vext."""
        pv = ps_sm.tile([128, 512], F32, tag="ps_sm", name=f"pv{tb}")
        for eb in range(EB):
            nc.tensor.matmul(pv[:], xT[:, eb, tb * 128:(tb + 1) * 128],
                             wv_sb[:, eb, :],
                             start=(eb == 0), stop=(eb == EB - 1))
        nc.vector.tensor_copy(
            vext[:, tb, :, 0:DH],
            pv[:].rearrange("p (h d) -> p h d", h=HPC))

    # ---- attention stream over super-keys (pair, sc) x t-blocks ----
    def normalize(acc, h, sc):
        """acc [s,4,65] -> concat[:, 4sc+i, h*64:(h+1)*64]"""
        r_t = nrm.tile([128, 4], F32, tag="r", name=f"r{h}_{sc}")
        nc.vector.reciprocal(r_t[:], acc[:, :, DH:DH + 1])
        for i in range(4):
            nc.vector.tensor_scalar_mul(
                concat[:, 4 * sc + i, h * DH:(h + 1) * DH],
                acc[:, i, 0:DH], r_t[:, i:i + 1])

    def transpose_chunk(sb):
        """concat[:, sb, :] -> concatT[:, :, sb*128:+128]"""
        pt = ps_sm.tile([128, 512], BF16, tag="ps_sm", name=f"pt{sb}")
        for fb in range(4):
            nc.tensor.transpose(pt[:, fb * 128:(fb + 1) * 128],
                                concat[:, sb, fb * 128:(fb + 1) * 128],
                                ident[:])
        nc.vector.tensor_copy(
            concatT[:, :, sb * 128:(sb + 1) * 128],
            pt[:].rearrange("p (fb s) -> p fb s", fb=4))

    def out_chunk(sb, ec):
        po = ps_sm.tile([128, 512], F32, tag="ps_sm", name=f"po{sb}_{ec}")
        for fb in range(4):
            nc.tensor.matmul(po[:],
                             concatT[:, fb, sb * 128:(sb + 1) * 128],
                             wo_sb[:, fb, ec * 512:(ec + 1) * 512],
                             start=(fb == 0), stop=(fb == 3))
        ot = outp.tile([128, 512], F32, tag="ot", name=f"ot{sb}_{ec}")
        nc.vector.tensor_copy(ot[:], po[:])
        nc.sync.dma_start(
            out_d[sb * 128:(sb + 1) * 128, ec * 512:(ec + 1) * 512], ot[:])

    # Filler queue: (deadline_slot, pe_cycles, closure).  Deadline-forced
    # pops keep hard deps satisfied; budget pops keep the PE busy at the
    # ACT cadence.  Queue order respects intra-queue dependencies.
    fillers = []

    def run_stream():
        keys = [(p, sc) for p in range(4) for sc in range(SC)]
        debt = [0]
        slot = [0]

        def inject():
            debt[0] += SLOT_BUDGET - STREAM_COST
            while fillers and (fillers[0][0] <= slot[0]
                               or fillers[0][1] <= debt[0]):
                _, cost, fn = fillers.pop(0)
                debt[0] -= cost
                fn()
            debt[0] = min(debt[0], 3 * SLOT_BUDGET)

        pend = [None]

        def flush_pend():
            if pend[0] is None:
                return
            ex, p, sc, tb, acc0, acc1 = pend[0]
            for hl in range(2):
                acc = acc0 if hl == 0 else acc1
                for i in range(4):
                    nc.tensor.matmul(
                        acc[:, i, :],
                        ex[:, hl * 512 + i * 128: hl * 512 + (i + 1) * 128],
                        vext[:, tb, 2 * p + hl, :],
                        start=(tb == 0), stop=(tb == SB - 1))
            if tb == SB - 1:
                normalize(acc0, 2 * p, sc)
                normalize(acc1, 2 * p + 1, sc)
                if p == 3:
                    for sb in range(4 * sc, 4 * sc + 4):
                        fillers.append((10 ** 9, 512,
                                        lambda sb=sb: transpose_chunk(sb)))
                        for ec in range(2):
                            fillers.append(
                                (10 ** 9, 2048,
                                 lambda sb=sb, ec=ec: out_chunk(sb, ec)))
            pend[0] = None

        for p, sc in keys:
            qT, kT = pair_qk[p]
            acc0 = ps_ac.tile([128, 4, DH + 1], F32, tag="acc0",
                              name=f"a0_{p}_{sc}")
            acc1 = ps_ac.tile([128, 4, DH + 1], F32, tag="acc1",
                              name=f"a1_{p}_{sc}")
            qs0 = qT[0:64, sc * 512:(sc + 1) * 512]
            qs1 = qT[64:128, sc * 512:(sc + 1) * 512]
            for tb in range(SB):
                scp = ps_sc.tile([128, 1024], F32, tag="scp",
                                 name=f"s{p}_{sc}_{tb}")
                nc.tensor.matmul(scp[:, 0:512],
                                 kT[0:64, tb * 128:(tb + 1) * 128], qs0,
                                 start=True, stop=True)
                nc.tensor.matmul(scp[:, 512:1024],
                                 kT[64:128, tb * 128:(tb + 1) * 128], qs1,
                                 start=True, stop=True)
                ex = exp_p.tile([128, 1024], BF16, tag="ex",
                                name=f"e{p}_{sc}_{tb}")
                nc.scalar.activation(ex[:], scp[:], Exp, scale=0.125)
                flush_pend()
                pend[0] = (ex, p, sc, tb, acc0, acc1)
                slot[0] += 1
                inject()
        flush_pend()
        for _, _, fn in fillers:
            fn()
        fillers.clear()

    # ---- DMA issue order: pair-0 weights + first xT chunks first ----
    dma_pair_weights(0)
    alloc_pair_qk(0)
    for eb in range(EB):
        dma_xt(eb, 0)
    for eb in range(EB):
        dma_xt(eb, 1)
    dma_wv()
    for sc in (2, 3):
        for eb in range(EB):
            dma_xt(eb, sc)
    dma_wo()

    # ---- filler schedule ----
    # v.tb(j): consumed by attn-V at slot j+1 -> deadline j.
    # k0.sc(j): needed by scores tb=4j -> deadline 4j-1.
    # q0.sc(j): needed at super-key (0, j) -> deadline 16j-1.
    # pair p>=1: weights DMA at pair p-1 start; chunks spread with
    # deadlines before first use at slot 64p (+4j for k, +16j for q).
    for j in range(1, SC):
        fillers.append((4 * j - 1, 4096, lambda j=j: proj_chunk(0, "k", j)))
    for j in range(SB):
        fillers.append((j, 4096, lambda j=j: v_chunk(j)))
    for j in range(1, SC):
        fillers.append((16 * j - 1, 4096, lambda j=j: proj_chunk(0, "q", j)))
    for p in range(1, 4):
        fillers.append((64 * (p - 1) + 8, 0, lambda p=p: (
            dma_pair_weights(p), alloc_pair_qk(p))))
        for j in range(SC):
            fillers.append((64 * p + 4 * j - 6, 4096,
                            lambda p=p, j=j: proj_chunk(p, "k", j)))
            fillers.append((64 * p + 16 * j - 3, 4096,
                            lambda p=p, j=j: proj_chunk(p, "q", j)))
    fillers.sort(key=lambda t: t[0])

    # ---- prefix: pair-0 sc0 projections, then the stream ----
    proj_chunk(0, "q", 0)
    proj_chunk(0, "k", 0)
    run_stream()


_CACHE = {}


def _build():
    nc = bacc.Bacc("TRN2", target_bir_lowering=False, debug=False,
                   num_devices=N_CORES)
    xt_d = nc.dram_tensor("xt", [128, EB * S], BF16, kind="ExternalInput").ap()
    wqk_d = nc.dram_tensor("wqk", [8 * 128, EB * 128], BF16,
                           kind="ExternalInput").ap()
    wv_d = nc.dram_tensor("wv", [128, EB * JW], BF16,
                          kind="ExternalInput").ap()
    wo_d = nc.dram_tensor("wo", [128, 4 * E], BF16,
                          kind="ExternalInput").ap()
    bq_d = nc.dram_tensor("bqt", [128, 4], F32, kind="ExternalInput").ap()
    id_d = nc.dram_tensor("ident", [128, 128], BF16,
                          kind="ExternalInput").ap()
    out_d = nc.dram_tensor("out", [S, E], F32, kind="ExternalOutput").ap()
    aps = (xt_d, wqk_d, wv_d, wo_d, bq_d, id_d, out_d)
    with tile.TileContext(nc) as tc:
        with ExitStack() as ctx:
            _emit(tc, aps, ctx)
    nc.compile()
    return nc


def _to_sbuf_layout(a, blocks, width):
    """[blocks*128, width] -> [128, blocks*width] (partition-major)."""
    return np.ascontiguousarray(
        a.reshape(blocks, 128, width).transpose(1, 0, 2).reshape(
            128, blocks * width))


def kernel(x, Wq, bq, Wk, bk, Wv, bv, Wo, bo):
    x = np.asarray(x, dtype=np.float32)
    Wq = np.asarray(Wq, dtype=np.float32)
    bq = np.asarray(bq, dtype=np.float32)
    Wk = np.asarray(Wk, dtype=np.float32)
    Wv = np.asarray(Wv, dtype=np.float32)
    bv = np.asarray(bv, dtype=np.float32)
    Wo = np.asarray(Wo, dtype=np.float32)
    bo = np.asarray(bo, dtype=np.float32)

    if "nc" not in _CACHE:
        _CACHE["nc"] = _build()
    nc = _CACHE["nc"]

    bf16 = ml_dtypes.bfloat16
    WoT = np.ascontiguousarray(Wo.T)  # [f, e]
    # bv and bo contributions, added on the host (exact: they commute
    # through the attention average / are affine in the output).
    bias_vec = bv.reshape(E) @ Wo.T + bo  # [E]

    in_maps = []
    for c in range(N_CORES):
        b, hh = c // 2, c % 2
        hs = slice(hh * HPC, (hh + 1) * HPC)
        wq2 = Wq[hs].transpose(1, 0, 2).reshape(E, JW)   # [e, (h d)]
        wk2 = Wk[hs].transpose(1, 0, 2).reshape(E, JW)
        wv2 = Wv[hs].transpose(1, 0, 2).reshape(E, JW)
        wqk = np.empty((8, 128, EB * 128), dtype=bf16)
        for p in range(4):
            wqk[p] = _to_sbuf_layout(
                wq2[:, p * 128:(p + 1) * 128], EB, 128).astype(bf16)
            wqk[4 + p] = _to_sbuf_layout(
                wk2[:, p * 128:(p + 1) * 128], EB, 128).astype(bf16)
        wqk = wqk.reshape(8 * 128, EB * 128)
        in_maps.append({
            "xt": _to_sbuf_layout(
                np.ascontiguousarray(x[b].T), EB, S).astype(bf16),
            "wqk": wqk,
            "wv": _to_sbuf_layout(wv2, EB, JW).astype(bf16),
            "wo": _to_sbuf_layout(
                WoT[hh * JW:(hh + 1) * JW], 4, E).astype(bf16),
            "bqt": np.ascontiguousarray(bq[hs].reshape(4, 128).T),
            "ident": np.eye(128, dtype=np.float32).astype(bf16),
        })

    res = bass_utils.run_bass_kernel_spmd(nc, in_maps,
                                          core_ids=list(range(N_CORES)))
    out = np.empty((B, S, E), dtype=np.float32)
    for b in range(B):
        out[b] = res.results[2 * b]["out"] + res.results[2 * b + 1]["out"]
        out[b] += bias_vec[None, :]
    return out


# revision 22
# speedup vs baseline: 1.1356x; 1.1356x over previous
"""Multi-head attention kernel for Trainium2, 8 NeuronCores.

Problem (hardcoded shapes): B=4, S=2048, E=1024, H=16, DH=64.
  q/k/v = einsum('bse,hed->bhsd', x, W{q,k,v}) + b{q,k,v}
  attn  = softmax(q k^T / sqrt(DH)) v
  out   = concat_heads(attn) @ Wo^T + bo

Sharding: core c -> (batch b = c//2, head-half hh = c%2, i.e. heads
8*hh..8*hh+7).  Each core computes a [S, E] partial of its batch's output
(its 512 columns of concat against the matching 512 rows of Wo^T); the host
sums the two partials per batch and adds the bias vector.

Bias algebra (exactness): softmax over keys t of
  (q_s+bq)·(k_t+bk) = q_s·k_t + bq·k_t + [q_s·bk + bq·bk]
The bracketed terms are constant in t and cancel in softmax, so bk is
dropped entirely; bq is folded into the qT copy.  bv commutes through the
attention average, so bv's contribution to the output is the constant
vector bv_cat @ Wo^T, added on the host together with bo.

Per-core dataflow (all PE inputs bf16; PSUM accumulation fp32):
  xT   [e=128 x 8, s=2048]  host-pretransposed, DMA'd directly
  qT/kT[j=128, s=2048]      per pair p: Wp^T @ xT (+bq on the q copy)
  v    [t, (h,65)]          xT @ Wv, with a fused ones column per head
  scores  [t=128, 1024]     per (pair, sc, tb): two matmuls, one per head
                            (cols 0:512 head 2p, 512: head 2p+1), both rhs
                            qT[*, sc*512:+512]
  ex                        ACT Exp(scale=1/8) -> bf16 SBUF
  attn+sums [s=128, 65]     lhsT = ex column-block (N=65!), rhs = v slice,
                            accumulated over the 16 t-blocks per (h, sb)
  normalize                 DVE recip + per-partition tensor_scalar_mul
                            -> concat [s, f] bf16
  concatT [f, s]            PE transpose (bf16), after all heads of an sb
  out_partial [s, e]        concatT as lhsT, Wo^T as rhs, DMA'd from PSUM

The stream is one software-pipelined generator over (pair, sc, tb) with
scores->exp one slot ahead of attn-V, and fillers (projection chunks, v
blocks, output chunks) injected on an ACT-cadence cycle budget with
deadline forcing, so both the PE and the scalar engine stay busy.
"""

import os
import sys

for _p in ("/opt/trn_rl_repo", "/root/.axon_site/_ro/trn_rl_repo"):
    if os.path.isdir(_p) and _p not in sys.path:
        sys.path.insert(0, _p)
        break

from contextlib import ExitStack

import numpy as np
import ml_dtypes

import concourse.bass as bass
import concourse.tile as tile
import concourse.mybir as mybir
from concourse import bacc, bass_utils

B, S, E, H, DH = 4, 2048, 1024, 16, 64
HPC = 8           # heads per core
JW = HPC * DH     # 512, per-core qkv width
N_CORES = 8
SB = S // 128     # 16 s-blocks / t-blocks
EB = E // 128     # 8 e-blocks
SC = S // 512     # 4 s-chunks
F32 = mybir.dt.float32
BF16 = mybir.dt.bfloat16
Exp = mybir.ActivationFunctionType.Exp
MULT = mybir.AluOpType.mult

# ACT slot cadence in PE cycles: one [128,1024] exp takes ~1038ns busy
# (~2600 PE cycles at 2.4GHz); fillers are injected against this budget.
SLOT_BUDGET = 2600
STREAM_COST = 1544      # scores (1024) + attn-V (520) per slot


def _emit(tc, aps, ctx):
    nc = tc.nc
    xt_d, wqk_d, wv_d, wo_d, bq_d, id_d, out_d = aps

    def pool(**kw):
        return ctx.enter_context(tc.tile_pool(**kw))

    const = pool(name="const", bufs=1)
    xTp = pool(name="xT", bufs=1)
    vxp = pool(name="vext", bufs=1)
    wqk = pool(name="wqk", bufs=2)
    qkp = pool(name="qk", bufs=2)
    exp_p = pool(name="ex", bufs=3)
    ccp = pool(name="cc", bufs=1)
    nrm = pool(name="nrm", bufs=2)
    outp = pool(name="outs", bufs=3)
    ps_sc = pool(name="ps_sc", bufs=2, space="PSUM")   # scores [128,1024] x2 = 4 banks
    ps_ac = pool(name="ps_ac", bufs=1, space="PSUM")   # acc [128,4,65] x2 tags = 2 banks
    ps_sm = pool(name="ps_sm", bufs=2, space="PSUM")   # proj/outs/transposes = 2 banks

    # ---- constants ----
    ident = const.tile([128, 128], BF16)
    nc.sync.dma_start(ident[:], id_d[:])
    bq_sb = const.tile([128, 4], F32)
    nc.sync.dma_start(bq_sb[:], bq_d[:])
    wv_sb = const.tile([128, EB, JW], BF16)
    wo_sb = const.tile([128, 4, E], BF16)

    xT = xTp.tile([128, EB, S], BF16)
    vext = vxp.tile([128, SB, HPC, DH + 1], BF16)
    nc.gpsimd.memset(vext[:, :, :, DH:DH + 1], 1.0)
    concat = ccp.tile([128, SB, JW], BF16)    # [s, (sb), f]
    concatT = ccp.tile([128, 4, S], BF16)     # [f, (fb), s]

    def dma_xt(eb, sc):
        nc.sync.dma_start(xT[:, eb, sc * 512:(sc + 1) * 512],
                          xt_d[:, eb * S + sc * 512: eb * S + (sc + 1) * 512])

    def dma_wv():
        nc.sync.dma_start(
            wv_sb[:].rearrange("p eb j -> p (eb j)"), wv_d[:])

    def dma_wo():
        nc.sync.dma_start(
            wo_sb[:].rearrange("p fb e -> p (fb e)"), wo_d[:])

    pair_w = {}

    def dma_wq(p):
        wq_t = wqk.tile([128, EB, 128], BF16, tag="wq")
        nc.sync.dma_start(wq_t[:].rearrange("p eb j -> p (eb j)"),
                          wqk_d[p * 128:(p + 1) * 128, :])
        return wq_t

    def dma_wk(p):
        wk_t = wqk.tile([128, EB, 128], BF16, tag="wk")
        nc.sync.dma_start(wk_t[:].rearrange("p eb j -> p (eb j)"),
                          wqk_d[(4 + p) * 128:(5 + p) * 128, :])
        return wk_t

    def dma_pair_weights(p):
        pair_w[p] = (dma_wq(p), dma_wk(p))

    pair_qk = {}

    def alloc_pair_qk(p):
        qT = qkp.tile([128, S], BF16, tag="qT")
        kT = qkp.tile([128, S], BF16, tag="kT")
        pair_qk[p] = (qT, kT)

    def proj_chunk(p, which, sc):
        """q or k projection for pair p, s-chunk sc: 8 matmuls + copy."""
        wq_t, wk_t = pair_w[p]
        qT, kT = pair_qk[p]
        w_t, dst = (wq_t, qT) if which == "q" else (wk_t, kT)
        pq = ps_sm.tile([128, 512], F32, tag="ps_sm",
                        name=f"p{which}{p}_{sc}")
        for eb in range(EB):
            nc.tensor.matmul(pq[:], w_t[:, eb, :],
                             xT[:, eb, sc * 512:(sc + 1) * 512],
                             start=(eb == 0), stop=(eb == EB - 1))
        if which == "q":
            nc.vector.tensor_scalar_add(
                dst[:, sc * 512:(sc + 1) * 512], pq[:], bq_sb[:, p:p + 1])
        else:
            nc.vector.tensor_copy(dst[:, sc * 512:(sc + 1) * 512], pq[:])

    def v_chunk(tb, hp):
        """v projection for t-block tb, head-pair hp (2 heads) into vext."""
        pv = ps_sm.tile([128, 128], F32, tag="ps_sm", name=f"pv{tb}_{hp}")
        for eb in range(EB):
            nc.tensor.matmul(pv[:], xT[:, eb, tb * 128:(tb + 1) * 128],
                             wv_sb[:, eb, hp * 128:(hp + 1) * 128],
                             start=(eb == 0), stop=(eb == EB - 1))
        nc.vector.tensor_copy(
            vext[:, tb, 2 * hp:2 * hp + 2, 0:DH],
            pv[:].rearrange("p (h d) -> p h d", h=2))

    # ---- attention stream over super-keys (pair, sc) x t-blocks ----
    def normalize(acc, h, sc):
        """acc [s,4,65] -> concat[:, 4sc+i, h*64:(h+1)*64]"""
        r_t = nrm.tile([128, 4], F32, tag="r", name=f"r{h}_{sc}")
        nc.vector.reciprocal(r_t[:], acc[:, :, DH:DH + 1])
        for i in range(4):
            nc.vector.tensor_scalar_mul(
                concat[:, 4 * sc + i, h * DH:(h + 1) * DH],
                acc[:, i, 0:DH], r_t[:, i:i + 1])

    def transpose_chunk(sb):
        """concat[:, sb, :] -> concatT[:, :, sb*128:+128]"""
        pt = ps_sm.tile([128, 512], BF16, tag="ps_sm", name=f"pt{sb}")
        for fb in range(4):
            nc.tensor.transpose(pt[:, fb * 128:(fb + 1) * 128],
                                concat[:, sb, fb * 128:(fb + 1) * 128],
                                ident[:])
        nc.vector.tensor_copy(
            concatT[:, :, sb * 128:(sb + 1) * 128],
            pt[:].rearrange("p (fb s) -> p fb s", fb=4))

    def out_chunk(sb, ec, alt=False):
        # alt: tail-only — borrow the (now idle) scores pool so more output
        # chunks can be in flight after the last exp.
        if alt:
            po = ps_sc.tile([128, 1024], F32, tag="scp",
                            name=f"po{sb}_{ec}")[:, 0:512]
        else:
            po = ps_sm.tile([128, 512], F32, tag="ps_sm",
                            name=f"po{sb}_{ec}")
        for fb in range(4):
            nc.tensor.matmul(po[:],
                             concatT[:, fb, sb * 128:(sb + 1) * 128],
                             wo_sb[:, fb, ec * 512:(ec + 1) * 512],
                             start=(fb == 0), stop=(fb == 3))
        ot = outp.tile([128, 512], F32, tag="ot", name=f"ot{sb}_{ec}")
        nc.vector.tensor_copy(ot[:], po[:])
        nc.sync.dma_start(
            out_d[sb * 128:(sb + 1) * 128, ec * 512:(ec + 1) * 512], ot[:])

    # Filler queue: (deadline_slot, pe_cycles, closure).  Deadline-forced
    # pops keep hard deps satisfied; budget pops keep the PE busy at the
    # ACT cadence.  Queue order respects intra-queue dependencies.
    fillers = []

    def run_stream():
        keys = [(p, sc) for p in range(4) for sc in range(SC)]
        debt = [0]
        slot = [0]

        def inject():
            debt[0] += SLOT_BUDGET - STREAM_COST
            while fillers and (fillers[0][0] <= slot[0]
                               or fillers[0][1] <= debt[0]):
                _, cost, fn = fillers.pop(0)
                debt[0] -= cost
                fn()
            debt[0] = min(debt[0], 3 * SLOT_BUDGET)

        pend = [None]

        def flush_pend():
            # acc is [128, 2(hl), 4(i), 128]: each hl-half is one PSUM bank
            # = one zero region = ONE accumulation group (start marks the
            # whole region pending-zero; first write per address overwrites,
            # later writes accumulate), so start only on the first matmul of
            # the half and stop on the last.
            if pend[0] is None:
                return
            ex, p, sc, tb, acc = pend[0]
            for hl in range(2):
                for i in range(4):
                    nc.tensor.matmul(
                        acc[:, hl, i, 0:DH + 1],
                        ex[:, hl * 512 + i * 128: hl * 512 + (i + 1) * 128],
                        vext[:, tb, 2 * p + hl, :],
                        start=(tb == 0 and i == 0),
                        stop=(tb == SB - 1 and i == 3))
            if tb == SB - 1:
                normalize(acc[:, 0], 2 * p, sc)
                normalize(acc[:, 1], 2 * p + 1, sc)
                if p == 3:
                    for sb in range(4 * sc, 4 * sc + 4):
                        fillers.append((10 ** 9, 512,
                                        lambda sb=sb: transpose_chunk(sb)))
                        for ec in range(2):
                            alt = (sc == 3 and ec == 1)
                            fillers.append(
                                (10 ** 9, 2048,
                                 lambda sb=sb, ec=ec, alt=alt:
                                 out_chunk(sb, ec, alt)))
            pend[0] = None

        for p, sc in keys:
            qT, kT = pair_qk[p]
            acc = ps_ac.tile([128, 2, 4, 128], F32, tag="acc",
                             name=f"a_{p}_{sc}")
            qs0 = qT[0:64, sc * 512:(sc + 1) * 512]
            qs1 = qT[64:128, sc * 512:(sc + 1) * 512]
            for tb in range(SB):
                scp = ps_sc.tile([128, 1024], F32, tag="scp",
                                 name=f"s{p}_{sc}_{tb}")
                nc.tensor.matmul(scp[:, 0:512],
                                 kT[0:64, tb * 128:(tb + 1) * 128], qs0,
                                 start=True, stop=True)
                nc.tensor.matmul(scp[:, 512:1024],
                                 kT[64:128, tb * 128:(tb + 1) * 128], qs1,
                                 start=True, stop=True)
                ex = exp_p.tile([128, 1024], BF16, tag="ex",
                                name=f"e{p}_{sc}_{tb}")
                nc.scalar.activation(ex[:], scp[:], Exp, scale=0.125)
                flush_pend()
                pend[0] = (ex, p, sc, tb, acc)
                slot[0] += 1
                inject()
        flush_pend()
        for _, _, fn in fillers:
            fn()
        fillers.clear()

    # ---- DMA issue order: wq0, the xT slices it needs, then emit the
    # first projection immediately so the PE starts as early as possible ----
    wq0 = dma_wq(0)
    for eb in range(EB):
        dma_xt(eb, 0)
    pair_w[0] = (wq0, None)
    alloc_pair_qk(0)
    proj_chunk(0, "q", 0)
    wk0 = dma_wk(0)
    pair_w[0] = (wq0, wk0)
    proj_chunk(0, "k", 0)
    dma_wv()
    for eb in range(EB):
        dma_xt(eb, 1)
    for sc in (2, 3):
        for eb in range(EB):
            dma_xt(eb, sc)
    dma_wo()

    # ---- filler schedule ----
    # v.tb(j) head-pair 0: consumed by attn-V at slot j+1 -> deadline j.
    # v head-pairs 1..3: needed at super-key (hp, 0) = slot 64*hp; spread
    # across the preceding sweep (deadline 64*(hp-1) + 16 + 3j).
    # k0.sc(j): needed by scores tb=4j -> deadline 4j-1.
    # q0.sc(j): needed at super-key (0, j) -> deadline 16j-1.
    # pair p>=1: weights DMA early in pair p-1's sweep; all 8 chunks done
    # by the end of that sweep (deadlines spread 64(p-1)+16 .. +62).
    for j in range(1, SC):
        fillers.append((4 * j - 1, 4096, lambda j=j: proj_chunk(0, "k", j)))
    for j in range(SB):
        fillers.append((j, 1024, lambda j=j: v_chunk(j, 0)))
    for j in range(1, SC):
        fillers.append((16 * j - 2, 4096, lambda j=j: proj_chunk(0, "q", j)))
    for p in range(1, 4):
        base = 64 * (p - 1)
        fillers.append((base + 6, 0, lambda p=p: (
            dma_pair_weights(p), alloc_pair_qk(p))))
        for j in range(SC):
            fillers.append((base + 30 + 8 * j, 4096,
                            lambda p=p, j=j: proj_chunk(p, "k", j)))
            fillers.append((64 * p - 3 if j == 0 else 64 * p + 16 * j - 2,
                            4096, lambda p=p, j=j: proj_chunk(p, "q", j)))
        for j in range(SB):
            fillers.append((base + 14 + 3 * j, 1024,
                            lambda p=p, j=j: v_chunk(j, p)))
    fillers.sort(key=lambda t: t[0])

    run_stream()


_CACHE = {}


def _build():
    nc = bacc.Bacc("TRN2", target_bir_lowering=False, debug=False,
                   num_devices=N_CORES)
    xt_d = nc.dram_tensor("xt", [128, EB * S], BF16, kind="ExternalInput").ap()
    wqk_d = nc.dram_tensor("wqk", [8 * 128, EB * 128], BF16,
                           kind="ExternalInput").ap()
    wv_d = nc.dram_tensor("wv", [128, EB * JW], BF16,
                          kind="ExternalInput").ap()
    wo_d = nc.dram_tensor("wo", [128, 4 * E], BF16,
                          kind="ExternalInput").ap()
    bq_d = nc.dram_tensor("bqt", [128, 4], F32, kind="ExternalInput").ap()
    id_d = nc.dram_tensor("ident", [128, 128], BF16,
                          kind="ExternalInput").ap()
    out_d = nc.dram_tensor("out", [S, E], F32, kind="ExternalOutput").ap()
    aps = (xt_d, wqk_d, wv_d, wo_d, bq_d, id_d, out_d)
    with tile.TileContext(nc) as tc:
        with ExitStack() as ctx:
            _emit(tc, aps, ctx)
    nc.compile()
    return nc


def _to_sbuf_layout(a, blocks, width):
    """[blocks*128, width] -> [128, blocks*width] (partition-major)."""
    return np.ascontiguousarray(
        a.reshape(blocks, 128, width).transpose(1, 0, 2).reshape(
            128, blocks * width))


def kernel(x, Wq, bq, Wk, bk, Wv, bv, Wo, bo):
    x = np.asarray(x, dtype=np.float32)
    Wq = np.asarray(Wq, dtype=np.float32)
    bq = np.asarray(bq, dtype=np.float32)
    Wk = np.asarray(Wk, dtype=np.float32)
    Wv = np.asarray(Wv, dtype=np.float32)
    bv = np.asarray(bv, dtype=np.float32)
    Wo = np.asarray(Wo, dtype=np.float32)
    bo = np.asarray(bo, dtype=np.float32)

    if "nc" not in _CACHE:
        _CACHE["nc"] = _build()
    nc = _CACHE["nc"]

    bf16 = ml_dtypes.bfloat16
    WoT = np.ascontiguousarray(Wo.T)  # [f, e]
    # bv and bo contributions, added on the host (exact: they commute
    # through the attention average / are affine in the output).
    bias_vec = bv.reshape(E) @ Wo.T + bo  # [E]

    in_maps = []
    for c in range(N_CORES):
        b, hh = c // 2, c % 2
        hs = slice(hh * HPC, (hh + 1) * HPC)
        wq2 = Wq[hs].transpose(1, 0, 2).reshape(E, JW)   # [e, (h d)]
        wk2 = Wk[hs].transpose(1, 0, 2).reshape(E, JW)
        wv2 = Wv[hs].transpose(1, 0, 2).reshape(E, JW)
        wqk = np.empty((8, 128, EB * 128), dtype=bf16)
        for p in range(4):
            wqk[p] = _to_sbuf_layout(
                wq2[:, p * 128:(p + 1) * 128], EB, 128).astype(bf16)
            wqk[4 + p] = _to_sbuf_layout(
                wk2[:, p * 128:(p + 1) * 128], EB, 128).astype(bf16)
        wqk = wqk.reshape(8 * 128, EB * 128)
        in_maps.append({
            "xt": _to_sbuf_layout(
                np.ascontiguousarray(x[b].T), EB, S).astype(bf16),
            "wqk": wqk,
            "wv": _to_sbuf_layout(wv2, EB, JW).astype(bf16),
            "wo": _to_sbuf_layout(
                WoT[hh * JW:(hh + 1) * JW], 4, E).astype(bf16),
            "bqt": np.ascontiguousarray(bq[hs].reshape(4, 128).T),
            "ident": np.eye(128, dtype=np.float32).astype(bf16),
        })

    res = bass_utils.run_bass_kernel_spmd(nc, in_maps,
                                          core_ids=list(range(N_CORES)))
    out = np.empty((B, S, E), dtype=np.float32)
    for b in range(B):
        out[b] = res.results[2 * b]["out"] + res.results[2 * b + 1]["out"]
        out[b] += bias_vec[None, :]
    return out
